# revision 1
# baseline (speedup 1.0000x reference)
"""Trainium2 Bass kernel for nn_CBSA_9517647528038 (sparse landmark attention).

Math (per batch sample b, head h, dh=64, 784 patches + 1 cls token):
  inner   = x @ W_in                                  [785, 768] -> 12 heads x 64
  reps    = Pfull @ patches            (adaptive pool) [64, 64]
  attn_u  = exp(SCALE * reps @ patches^T)             [64, 784]  (unnormalized)
  reps'   = reps + step_rep * (attn_u @ patches) / rowsum(attn_u)
  attn2_u = exp(SCALE * reps' @ reps'^T)              [64, 64]   (symmetric!)
  rtr     = attn2_u @ reps' / rowsum(attn2_u)
  deltaT  = (step_x/rowsum(attn_u) * rtr)^T-contracted with attn_u  [64d, 784]
  y       = x + concat(cls, delta) @ W_out

Distribution: data-parallel over batch, 8 samples per core, 8 cores, no
collectives.  Dtypes: projections in float32r (fp32 with 11-bit mantissa,
full PE rate at N>=256), attention internals in fp16.  Softmax max-
subtraction is skipped (logits are O(0.3)), matching jax softmax up to fp
rounding.  All normalizations are folded into per-partition scale vectors.
"""

import numpy as np

HEADS = 12
DH = 64
REP = 8
GRID = 28
SCALE = DH ** -0.5
B, N, D = 64, 785, 768
NP = 786        # token dim padded to even (f32r matmul needs even N)
NC = 8          # cores
BS = B // NC    # samples per core
NPATCH = 784
NPAD = 896      # 7 * 128, zero padded patch dim
KT = 6          # 768 / 128 contraction tiles
NT = 7          # patch tiles of 128 (last has 16 valid rows)

_STATE = {}
import os
VARIANT = set(os.environ.get("CBSA_VARIANT", "").split(","))


def _pool_matrix(in_size, out_size):
    P = np.zeros((out_size, in_size), np.float32)
    for i in range(out_size):
        s = (i * in_size) // out_size
        e = -((-(i + 1) * in_size) // out_size)
        P[i, s:e] = 1.0 / (e - s)
    return P


def _build_program():
    import concourse.bass as bass
    import concourse.tile as tile
    import concourse.mybir as mybir
    from concourse import bacc

    f32 = mybir.dt.float32
    f32r = mybir.dt.float32r
    f16 = mybir.dt.float16
    AF = mybir.ActivationFunctionType

    nc = bacc.Bacc(trn_type="TRN2", target_bir_lowering=False, debug=False)

    # ---- DRAM I/O (per core) ----
    xT_d = nc.dram_tensor("xT", [BS, D, NP], f32r, kind="ExternalInput")
    win_d = nc.dram_tensor("win", [D, D], f32r, kind="ExternalInput")
    wout_d = nc.dram_tensor("wout", [D, D], f16, kind="ExternalInput")
    pft_d = nc.dram_tensor("pft", [NPAD, DH], f16, kind="ExternalInput")
    idr_d = nc.dram_tensor("idr", [128, 128], f32r, kind="ExternalInput")
    idh_d = nc.dram_tensor("idh", [128, 128], f16, kind="ExternalInput")
    srp_d = nc.dram_tensor("srp", [128, 6], f32, kind="ExternalInput")
    sxp_d = nc.dram_tensor("sxp", [128, 6], f32, kind="ExternalInput")
    yT_d = nc.dram_tensor("yT", [BS, D, NP], f32, kind="ExternalOutput")

    with tile.TileContext(nc) as tc:
        with (
            tc.tile_pool(name="const", bufs=1) as pc,
            tc.tile_pool(name="sample", bufs=2) as psmp,
            tc.tile_pool(name="attn", bufs=2) as pat,
            tc.tile_pool(name="small", bufs=3) as psm,
            tc.tile_pool(name="stage", bufs=3) as pst,
            tc.tile_pool(name="ppj", bufs=4, space="PSUM") as ppj,
            tc.tile_pool(name="psml", bufs=3, space="PSUM") as psml,
        ):
            # ---- constants ----
            win_sb = [pc.tile([128, D], f32r, tag=f"win{k}", name=f"win{k}") for k in range(KT)]
            wout_sb = [pc.tile([128, D], f16, tag=f"wout{k}", name=f"wout{k}") for k in range(KT)]
            pft_sb = [pc.tile([128, DH], f16, tag=f"pft{t}", name=f"pft{t}") for t in range(NT)]
            idr = pc.tile([128, 128], f32r, tag="idr", name="idr")
            idh = pc.tile([128, 128], f16, tag="idh", name="idh")
            srp = pc.tile([128, 6], f32, tag="srp", name="srp")
            sxp = pc.tile([128, 6], f32, tag="sxp", name="sxp")
            for k in range(KT):
                nc.sync.dma_start(win_sb[k][:], win_d[k * 128:(k + 1) * 128, :])
                nc.sync.dma_start(wout_sb[k][:], wout_d[k * 128:(k + 1) * 128, :])
            for t in range(NT):
                nc.sync.dma_start(pft_sb[t][:], pft_d[t * 128:(t + 1) * 128, :])
            nc.sync.dma_start(idr[:], idr_d[:])
            nc.sync.dma_start(idh[:], idh_d[:])
            nc.sync.dma_start(srp[:], srp_d[:])
            nc.sync.dma_start(sxp[:], sxp_d[:])

            cp = [0]  # alternating copy engine

            def evac(dst, src):
                """PSUM -> SBUF copy, alternating ACT/DVE."""
                cp[0] += 1
                if cp[0] % 2:
                    nc.scalar.copy(dst, src)
                else:
                    nc.vector.tensor_copy(dst, src)

            for s in range(BS):
                # ---- load xT (f32r bits == raw f32) ----
                xts = [psmp.tile([128, NP], f32r, tag=f"xt{k}", name=f"xt{k}") for k in range(KT)]
                for k in range(KT):
                    nc.sync.dma_start(xts[k][:], xT_d[s, k * 128:(k + 1) * 128, :])

                # ---- innerT = W_in^T @ x^T  -> fp16 [hd, token] ----
                inT = [psmp.tile([128, NP], f16, tag=f"inT{m}", name=f"inT{m}") for m in range(KT)]
                for m in range(KT):
                    for c0, c1 in ((0, 512), (512, NP)):
                        ps = ppj.tile([128, 512], f32, tag="proj", name="proj")
                        for k in range(KT):
                            nc.tensor.matmul(
                                ps[:, 0:c1 - c0],
                                win_sb[k][:, m * 128:(m + 1) * 128],
                                xts[k][:, c0:c1],
                                start=(k == 0), stop=(k == KT - 1),
                            )
                        evac(inT[m][:, c0:c1], ps[:, 0:c1 - c0])

                # ---- patches natural = x_patches @ W_in -> fp16 [n, hd] ----
                pnat = [psmp.tile([128, D], f16, tag=f"pn{t}", name=f"pn{t}") for t in range(NT)]
                nc.gpsimd.memset(pnat[NT - 1][:], 0.0)
                if "nopnat" in VARIANT:
                    for t in range(NT - 1):
                        nc.gpsimd.memset(pnat[t][:], 0.001)
                for t in range(NT if "nopnat" not in VARIANT else 0):
                    sz = 128 if t < NT - 1 else 16
                    for c0, c1 in ((0, 384), (384, 768)):
                        ps = ppj.tile([128, 512], f32, tag="proj", name="proj")
                        for k in range(KT):
                            nc.tensor.matmul(
                                ps[0:sz, 0:c1 - c0],
                                xts[k][:, 1 + t * 128: 1 + t * 128 + sz],
                                win_sb[k][:, c0:c1],
                                start=(k == 0), stop=(k == KT - 1),
                            )
                        evac(pnat[t][0:sz, c0:c1], ps[0:sz, 0:c1 - c0])

                # ---- pooling: reps_stack [128, 768] fp16 ----
                # rows 0:64  = head h cols h*64      (h = 0..11)
                # rows 64:128= head h cols (h-1)*64  (h = 1..11)
                rstk = psmp.tile([128, D], f16, tag="rstk", name="rstk")
                if "nopool" in VARIANT:
                    nc.gpsimd.memset(rstk[:], 0.001)
                for ci in range(0 if "nopool" in VARIANT else 2):
                    ps = ppj.tile([128, 512], f32, tag="proj", name="proj")
                    lo = (0, 384) if ci == 0 else (384, 768)
                    hi = (64, 448) if ci == 0 else (448, 768)
                    for t in range(NT):
                        nc.tensor.matmul(
                            ps[0:64, 0:384], pft_sb[t][:],
                            pnat[t][:, lo[0]:lo[1]],
                            start=(t == 0), stop=(t == NT - 1),
                        )
                    for t in range(NT):
                        nc.tensor.matmul(
                            ps[64:128, 0:hi[1] - hi[0]], pft_sb[t][:],
                            pnat[t][:, hi[0]:hi[1]],
                            start=(t == 0), stop=(t == NT - 1),
                        )
                    evac(rstk[:, ci * 384:(ci + 1) * 384], ps[:, 0:384])

                # ---- per head pair ----
                dT = [psmp.tile([128, NP], f16, tag=f"dT{m}", name=f"dT{m}") for m in range(KT)]
                if "noattn" in VARIANT:
                    for m in range(KT):
                        nc.gpsimd.memset(dT[m][:], 0.0)
                for hp in range(0 if "noattn" in VARIANT else KT):
                    he = 2 * hp
                    col = he * DH  # reps col offset for both halves of rstk

                    # repsT via PE transpose (both halves)
                    prT = psml.tile([128, DH], f16, tag="sm", name="sm")
                    nc.tensor.transpose(
                        prT[0:64, :], rstk[0:64, col:col + DH], idh[0:64, 0:64])
                    if "notp64" in VARIANT:
                        nc.tensor.transpose(
                            prT[64:128, :], rstk[0:64, col:col + DH],
                            idh[0:64, 0:64])
                    else:
                        nc.tensor.transpose(
                            prT[64:128, :], rstk[64:128, col:col + DH],
                            idh[64:128, 64:128])
                    repsT = psm.tile([128, DH], f16, tag="repsT", name="repsT")
                    evac(repsT[:], prT[:])

                    # logits [r, n] for the pair (two 1-bank psum chunks)
                    pls = []
                    for c0, c1 in ((0, 512), (512, NPATCH)):
                        pl = ppj.tile([128, 512], f32, tag="proj", name="proj")
                        pls.append(pl)
                        for p0, p1 in ((0, 64), (64, 128)):
                            nc.tensor.matmul(
                                pl[p0:p1, 0:c1 - c0], repsT[p0:p1, :],
                                inT[hp][p0:p1, 1 + c0:1 + c1],
                                start=True, stop=True,
                            )

                    # attn_u = exp(SCALE * logits), denom via accum
                    au = pat.tile([128, NPAD], f16, tag="au", name="au")
                    nc.gpsimd.memset(au[:, NPATCH:NPAD], 0.0)
                    d1 = psm.tile([128, 1], f32, tag="d1", name="d1")
                    d2 = psm.tile([128, 1], f32, tag="d2", name="d2")
                    nc.scalar.activation(au[:, 0:512], pls[0][:, 0:512], AF.Exp,
                                         scale=SCALE, accum_out=d1[:])
                    nc.scalar.activation(au[:, 512:NPATCH],
                                         pls[1][:, 0:NPATCH - 512],
                                         AF.Exp, scale=SCALE, accum_out=d2[:])
                    den = psm.tile([128, 1], f32, tag="den", name="den")
                    nc.vector.tensor_add(den[:], d1[:], d2[:])
                    rcp = psm.tile([128, 1], f32, tag="rcp", name="rcp")
                    nc.vector.reciprocal(rcp[:], den[:])

                    # attn_uT via PE transposes (DMA xbar transpose races
                    # with concurrent copy DMAs on this HW - known bug)
                    auT = pat.tile([128, NPAD], f16, tag="auT", name="auT")
                    for t in range(NT):
                        ptp = psml.tile([128, 128], f16, tag="sm", name="sm")
                        nc.tensor.transpose(
                            ptp[:], au[:, t * 128:(t + 1) * 128], idh[:])
                        evac(auT[:, t * 128:(t + 1) * 128], ptp[:])

                    # rep_delta (pair-fused, diag blocks valid)
                    prd = psml.tile([128, 128], f32, tag="sm", name="sm")
                    for t in range(NT):
                        nc.tensor.matmul(
                            prd[:], auT[:, t * 128:(t + 1) * 128],
                            pnat[t][:, col:col + 128],
                            start=(t == 0), stop=(t == NT - 1),
                        )

                    # reps' = reps + (step_rep * rcp) * rep_delta
                    svec = psm.tile([128, 1], f32, tag="svec", name="svec")
                    nc.vector.tensor_scalar(
                        svec[:], rcp[:], srp[:, hp:hp + 1], None,
                        op0=mybir.AluOpType.mult)
                    ru = psm.tile([128, DH], f16, tag="ru", name="ru")
                    for p0, p1, q0 in ((0, 64, 0), (64, 128, 64)):
                        nc.scalar.mul(ru[p0:p1, :], prd[p0:p1, q0:q0 + DH],
                                      svec[p0:p1, :])
                        nc.vector.tensor_add(ru[p0:p1, :], ru[p0:p1, :],
                                             rstk[p0:p1, col:col + DH])

                    # reps'^T
                    prT2 = psml.tile([128, DH], f16, tag="sm", name="sm")
                    nc.tensor.transpose(prT2[0:64, :], ru[0:64, :],
                                        idh[0:64, 0:64])
                    nc.tensor.transpose(prT2[64:128, :], ru[64:128, :],
                                        idh[64:128, 64:128])
                    ruT = psm.tile([128, DH], f16, tag="ruT", name="ruT")
                    evac(ruT[:], prT2[:])

                    # attn2_u = exp(SCALE * reps' @ reps'^T)  (symmetric)
                    pl2 = psml.tile([128, DH], f32, tag="sm", name="sm")
                    for p0, p1 in ((0, 64), (64, 128)):
                        nc.tensor.matmul(pl2[p0:p1, :], ruT[p0:p1, :],
                                         ruT[p0:p1, :], start=True, stop=True)
                    a2 = psm.tile([128, DH], f16, tag="a2", name="a2")
                    d3 = psm.tile([128, 1], f32, tag="d3", name="d3")
                    nc.scalar.activation(a2[:], pl2[:], AF.Exp, scale=SCALE,
                                         accum_out=d3[:])
                    rcp2 = psm.tile([128, 1], f32, tag="rcp2", name="rcp2")
                    nc.vector.reciprocal(rcp2[:], d3[:])

                    # rtr = attn2_u @ reps',  scaled by rcp*rcp2*step_x
                    prtr = psml.tile([128, DH], f32, tag="sm", name="sm")
                    for p0, p1 in ((0, 64), (64, 128)):
                        nc.tensor.matmul(prtr[p0:p1, :], a2[p0:p1, :],
                                         ru[p0:p1, :], start=True, stop=True)
                    s2 = psm.tile([128, 1], f32, tag="s2", name="s2")
                    nc.vector.tensor_scalar(
                        s2[:], rcp2[:], rcp[:], sxp[:, hp:hp + 1],
                        op0=mybir.AluOpType.mult, op1=mybir.AluOpType.mult)
                    rtrs = psm.tile([128, DH], f16, tag="rtrs", name="rtrs")
                    nc.vector.tensor_scalar(
                        rtrs[:], prtr[:], s2[:], None, op0=mybir.AluOpType.mult)

                    # delta_tokens^T [d-pair, n] (two 1-bank psum chunks)
                    for c0, c1 in ((0, 512), (512, NPATCH)):
                        pdt = ppj.tile([128, 512], f32, tag="proj", name="proj")
                        for p0, p1 in ((0, 64), (64, 128)):
                            nc.tensor.matmul(
                                pdt[p0:p1, 0:c1 - c0], rtrs[p0:p1, :],
                                au[p0:p1, c0:c1], start=True, stop=True)
                        evac(dT[hp][:, 1 + c0:1 + c1], pdt[:, 0:c1 - c0])
                    nc.vector.tensor_copy(dT[hp][:, 0:1], inT[hp][:, 0:1])

                # ---- y^T = W_out^T @ deltaT + x^T ----
                for m in range(KT):
                    for c0, c1 in ((0, 512), (512, NP)):
                        ps = ppj.tile([128, 512], f32, tag="proj", name="proj")
                        for k in range(KT):
                            nc.tensor.matmul(
                                ps[:, 0:c1 - c0],
                                wout_sb[k][:, m * 128:(m + 1) * 128],
                                dT[k][:, c0:c1], start=(k == 0),
                                stop=(k == KT - 1))
                        yst = pst.tile([128, 512], f32, tag="yst", name="yst")
                        xsl = xts[m][:, c0:c1].bitcast(f32)
                        cp[0] += 1
                        if cp[0] % 2:
                            nc.vector.tensor_add(
                                yst[:, 0:c1 - c0], ps[:, 0:c1 - c0], xsl)
                        else:
                            nc.scalar.copy(yst[:, 0:c1 - c0], ps[:, 0:c1 - c0])
                            nc.gpsimd.tensor_add(
                                yst[:, 0:c1 - c0], yst[:, 0:c1 - c0], xsl)
                        nc.sync.dma_start(
                            yT_d[s, m * 128:(m + 1) * 128, c0:c1],
                            yst[:, 0:c1 - c0])

    nc.finalize()
    return nc


def _get_state():
    if "nc" not in _STATE:
        _STATE["nc"] = _build_program()
    return _STATE["nc"]


def _host_inputs(x, W_in, W_out, step_x, step_rep):
    Ph = _pool_matrix(GRID, REP)
    Pfull = np.kron(Ph, Ph)                      # [64, 784]
    pft = np.zeros((NPAD, DH), np.float16)
    pft[:NPATCH, :] = Pfull.T.astype(np.float16)

    srp = np.zeros((128, 6), np.float32)
    sxp = np.zeros((128, 6), np.float32)
    sr = np.asarray(step_rep).reshape(HEADS)
    sx = np.asarray(step_x).reshape(HEADS)
    for hp in range(6):
        srp[0:64, hp] = sr[2 * hp]
        srp[64:128, hp] = sr[2 * hp + 1]
        sxp[0:64, hp] = sx[2 * hp]
        sxp[64:128, hp] = sx[2 * hp + 1]

    com = {
        "win": np.ascontiguousarray(W_in, np.float32),
        "wout": np.ascontiguousarray(W_out).astype(np.float16),
        "pft": pft,
        "idr": np.eye(128, dtype=np.float32),
        "idh": np.eye(128, dtype=np.float16),
        "srp": srp,
        "sxp": sxp,
    }
    xT = np.zeros((B, D, NP), np.float32)
    xT[:, :, 0:N] = np.transpose(x, (0, 2, 1))
    in_maps = []
    for c in range(NC):
        m = dict(com)
        m["xT"] = xT[c * BS:(c + 1) * BS]
        in_maps.append(m)
    return in_maps


def kernel(x, W_in, W_out, step_x, step_rep):
    from concourse import bass2jax
    nc = _get_state()
    in_maps = _host_inputs(x, W_in, W_out, step_x, step_rep)
    res = bass2jax.run_bass_via_pjrt(nc, in_maps, NC)
    y = np.empty((B, N, D), np.float32)
    for c in range(NC):
        y[c * BS:(c + 1) * BS] = np.transpose(res[c]["yT"][:, :, 0:N], (0, 2, 1))
    return y



# revision 14
# speedup vs baseline: 149.1374x; 149.1374x over previous
"""Trainium2 Bass kernel for nn_CBSA_9517647528038 (sparse landmark attention).

Math (per batch sample b, head h, dh=64, 784 patches + 1 cls token):
  inner   = x @ W_in                                  [785, 768] -> 12 heads x 64
  reps    = Pfull @ patches            (adaptive pool) [64, 64]
  attn_u  = exp(SCALE * reps @ patches^T)             [64, 784]  (unnormalized)
  reps'   = reps + step_rep * (attn_u @ patches) / rowsum(attn_u)
  attn2_u = exp(SCALE * reps' @ reps'^T)              [64, 64]   (symmetric!)
  rtr     = attn2_u @ reps' / rowsum(attn2_u)
  deltaT  = (step_x/rowsum(attn_u) * rtr)^T-contracted with attn_u  [64d, 784]
  y       = x + concat(cls, delta) @ W_out

Distribution: data-parallel over batch, 8 samples per core, 8 cores, no
collectives.  Dtypes: projections in float32r, attention internals in fp16.
Softmax max-subtraction is skipped (logits are O(0.3)).  All normalizations
are folded into per-partition scale vectors.

Schedule: the per-sample stages are software-pipelined at emission level so
the PE instruction stream (a per-engine FIFO) always has independent matmul
work between the serial attention dependency chains: iteration i emits
attention(sample i-1) breadth-first across the 6 head-pair chains,
interleaved with the projections of sample i, followed by the output
projection of sample i-1.

x and y travel in a partition-major tiled layout [128, KT*NP] per sample so
each sample is one large DMA; the host packs/unpacks.
"""

import numpy as np

HEADS = 12
DH = 64
REP = 8
GRID = 28
SCALE = DH ** -0.5
B, N, D = 64, 785, 768
NP = 786        # token dim padded to even (f32r matmul needs even N)
NC = 8          # cores
BS = B // NC    # samples per core
NPATCH = 784
NPAD = 896      # 7 * 128, zero padded patch dim
NP8 = 800       # fp8 tile row stride: DoubleRow k-dim step must be 16B-aligned
KT = 6          # 768 / 128 contraction tiles
NT = 7          # patch tiles of 128 (last has 16 valid rows)

_STATE = {}


def _pool_matrix(in_size, out_size):
    P = np.zeros((out_size, in_size), np.float32)
    for i in range(out_size):
        s = (i * in_size) // out_size
        e = -((-(i + 1) * in_size) // out_size)
        P[i, s:e] = 1.0 / (e - s)
    return P


def _build_program():
    import concourse.bass as bass
    import concourse.tile as tile
    import concourse.mybir as mybir
    from concourse import bacc

    f32 = mybir.dt.float32
    f16 = mybir.dt.float16
    f8 = mybir.dt.float8e4
    AF = mybir.ActivationFunctionType
    DR = mybir.MatmulPerfMode.DoubleRow

    nc = bacc.Bacc(trn_type="TRN2", target_bir_lowering=False, debug=False)

    # ---- DRAM I/O (per core) ----
    xT_d = nc.dram_tensor("xT", [BS, 128, KT * NP], f32, kind="ExternalInput")
    x8_d = nc.dram_tensor("x8", [BS, 128, KT, NP8], f8, kind="ExternalInput")
    win_d = nc.dram_tensor("win", [128, KT, D], f8, kind="ExternalInput")
    wout_d = nc.dram_tensor("wout", [128, KT, D], f8, kind="ExternalInput")
    pft_d = nc.dram_tensor("pft", [NPAD, DH], f16, kind="ExternalInput")
    idh_d = nc.dram_tensor("idh", [128, 128], f16, kind="ExternalInput")
    srp_d = nc.dram_tensor("srp", [128, 6], f32, kind="ExternalInput")
    sxp_d = nc.dram_tensor("sxp", [128, 6], f32, kind="ExternalInput")
    yT_d = nc.dram_tensor("yT", [BS, 128, KT * NP], f32, kind="ExternalOutput")

    with tile.TileContext(nc) as tc:
        with (
            tc.tile_pool(name="const", bufs=1) as pc,
            tc.tile_pool(name="sample", bufs=2) as psmp,
            tc.tile_pool(name="attn", bufs=7) as pat,
            tc.tile_pool(name="attnT", bufs=4) as patT,
            tc.tile_pool(name="small", bufs=7) as psm,
            tc.tile_pool(name="ppj", bufs=4, space="PSUM") as ppj,
            tc.tile_pool(name="psml", bufs=2, space="PSUM") as psml,
        ):
            # ---- constants ----
            win_sb = pc.tile([128, KT, D], f8, tag="win", name="win")
            wout_sb = pc.tile([128, KT, D], f8, tag="wout", name="wout")
            pft_sb = [pc.tile([128, DH], f16, tag=f"pft{t}", name=f"pft{t}") for t in range(NT)]
            idh = pc.tile([128, 128], f16, tag="idh", name="idh")
            srp = pc.tile([128, 6], f32, tag="srp", name="srp")
            sxp = pc.tile([128, 6], f32, tag="sxp", name="sxp")
            nc.sync.dma_start(win_sb[:], win_d[:])
            nc.sync.dma_start(wout_sb[:], wout_d[:])
            for t in range(NT):
                nc.sync.dma_start(pft_sb[t][:], pft_d[t * 128:(t + 1) * 128, :])
            nc.sync.dma_start(idh[:], idh_d[:])
            nc.sync.dma_start(srp[:], srp_d[:])
            nc.sync.dma_start(sxp[:], sxp_d[:])

            # cost-balancing router for ACT/DVE work (model: ns per op)
            acc = {'act': 0.0, 'dve': 0.0}

            def act_work(ns):
                acc['act'] += ns

            def dve_work(ns):
                acc['dve'] += ns

            def evac(dst, src, fast=False):
                """PSUM -> SBUF copy routed to the less-loaded of ACT/DVE.
                fast=True when both dtypes are 2-byte (DVE 2x mode)."""
                free = src.free_size()
                ca = (free * 0.833 + 143) * 1.18
                cd = free * (0.52 if fast else 1.04) + 125
                if acc['act'] + ca < acc['dve'] + cd:
                    acc['act'] += ca
                    nc.scalar.copy(dst, src)
                else:
                    acc['dve'] += cd
                    nc.vector.tensor_copy(dst, src)

            # ---------------- per-sample unit generators ----------------

            def proj_units(st):
                """Load x, project to inT (transposed) / pnat (natural),
                pool to reps.  Returns list of emission closures."""
                units = []

                def u_load():
                    st['x8'] = psmp.tile([128, KT, NP8], f8, tag="x8", name="x8")
                    nc.sync.dma_start(st['x8'][:], x8_d[st['s']])
                    # f32 copy of x straight into the output staging tile: the
                    # residual add then needs no x tile at output time (issued
                    # on the Pool DMA queue so it doesn't serialize with x8)
                    st['yst'] = psmp.tile([128, KT * NP], f32, tag="yst", name="yst")
                    nc.sync.dma_start(st['yst'][:], xT_d[st['s']])
                    st['inT'] = [psmp.tile([128, NP], f16, tag=f"inT{m}", name=f"inT{m}")
                                 for m in range(KT)]
                    st['pnat'] = [psmp.tile([128, D], f16, tag=f"pn{t}", name=f"pn{t}")
                                  for t in range(NT)]
                    st['dT8'] = psmp.tile([128, KT, NP8], f8, tag="dT8", name="dT8")
                    st['rstk'] = psmp.tile([128, D], f16, tag="rstk", name="rstk")
                    nc.gpsimd.memset(st['pnat'][NT - 1][:], 0.0)
                    nc.gpsimd.memset(st['dT8'][:, :, NP - 1:NP], 0.0)
                units.append(u_load)

                def mk_inT(m, c0, c1):
                    def u():
                        ps = ppj.tile([128, 512], f32, tag="proj", name="proj")
                        for k in range(0, KT, 2):
                            nc.tensor.matmul(
                                ps[:, 0:c1 - c0],
                                win_sb[:, k:k + 2, m * 128:(m + 1) * 128],
                                st['x8'][:, k:k + 2, c0:c1],
                                start=(k == 0), stop=(k == KT - 2),
                                perf_mode=DR,
                            )
                        evac(st['inT'][m][:, c0:c1], ps[:, 0:c1 - c0])
                    return u
                for m in range(KT):
                    for c0, c1 in ((0, 512), (512, NP)):
                        units.append(mk_inT(m, c0, c1))

                def mk_pnat(t, c0, c1):
                    sz = 128 if t < NT - 1 else 16
                    def u():
                        ps = ppj.tile([128, 512], f32, tag="proj", name="proj")
                        for k in range(0, KT, 2):
                            nc.tensor.matmul(
                                ps[0:sz, 0:c1 - c0],
                                st['x8'][:, k:k + 2, 1 + t * 128: 1 + t * 128 + sz],
                                win_sb[:, k:k + 2, c0:c1],
                                start=(k == 0), stop=(k == KT - 2),
                                perf_mode=DR,
                            )
                        evac(st['pnat'][t][0:sz, c0:c1], ps[0:sz, 0:c1 - c0])
                    return u
                for t in range(NT):
                    for c0, c1 in ((0, 384), (384, 768)):
                        units.append(mk_pnat(t, c0, c1))

                def mk_pool(lo0, lo1):
                    def u():
                        ps = ppj.tile([128, 512], f32, tag="proj", name="proj")
                        for t in range(NT):
                            nc.tensor.matmul(
                                ps[0:64, 0:384], pft_sb[t][:],
                                st['pnat'][t][:, lo0:lo1],
                                start=(t == 0), stop=(t == NT - 1),
                            )
                        evac(st['rstk'][0:64, lo0:lo1], ps[0:64, 0:384])
                    return u
                units.append(mk_pool(0, 384))
                units.append(mk_pool(384, 768))

                def u_stack():
                    # rows 64:128 hold the odd head of each pair: a shifted
                    # copy of rows 0:64 (cross-partition, so via DMA)
                    nc.scalar.dma_start(st['rstk'][64:128, 0:D - DH],
                                        st['rstk'][0:64, DH:D])
                units.append(u_stack)
                return units

            def attn_units(st):
                """Breadth-first attention across the 6 head-pair chains."""
                units = []
                ch = [dict() for _ in range(KT)]

                def mk_seg0(hp):
                    col = 2 * hp * DH
                    c = ch[hp]
                    def u():
                        # repsT pair [dh-pair, r] via one PE transpose
                        prT = psml.tile([128, DH], f16, tag="sm", name="sm")
                        nc.tensor.matmul(
                            prT[:], st['rstk'][0:64, col:col + 128],
                            idh[0:64, 0:64],
                            is_transpose=True, start=True, stop=True,
                            skip_group_check=True)
                        c['repsT'] = psm.tile([128, DH], f16, tag="repsT", name="repsT")
                        evac(c['repsT'][:], prT[:])
                    return u

                def mk_seg1(hp):
                    c = ch[hp]
                    def u():
                        # logits [r, n] (two 1-bank psum chunks) -> exp -> au
                        c['au'] = pat.tile([128, NPAD], f16, tag="au", name="au")
                        nc.gpsimd.memset(c['au'][:, NPATCH:NPAD], 0.0)
                        d1 = psm.tile([128, 1], f32, tag="d1", name="d1")
                        d2 = psm.tile([128, 1], f32, tag="d2", name="d2")
                        for ci, (c0, c1) in enumerate(((0, 512), (512, NPATCH))):
                            pl = ppj.tile([128, 512], f32, tag="proj", name="proj")
                            for p0, p1 in ((0, 64), (64, 128)):
                                nc.tensor.matmul(
                                    pl[p0:p1, 0:c1 - c0], c['repsT'][p0:p1, :],
                                    st['inT'][hp][p0:p1, 1 + c0:1 + c1],
                                    start=True, stop=True,
                                    skip_group_check=True,
                                )
                            act_work((c1 - c0) * 0.833 + 320)
                            nc.scalar.activation(
                                c['au'][:, c0:c1], pl[:, 0:c1 - c0], AF.Exp,
                                scale=SCALE, accum_out=(d1 if ci == 0 else d2)[:])
                        den = psm.tile([128, 1], f32, tag="den", name="den")
                        dve_work(260)
                        nc.vector.tensor_add(den[:], d1[:], d2[:])
                        c['rcp'] = psm.tile([128, 1], f32, tag="rcp", name="rcp")
                        nc.vector.reciprocal(c['rcp'][:], den[:])
                    return u

                def mk_seg2(hp):
                    c = ch[hp]
                    def u():
                        # attn_uT: 7 PE transposes into ONE psum bank
                        # ([128, 896] fp16 = 1792B < 2KB), single evac.
                        ptp = psml.tile([128, NPAD], f16, tag="smw", name="smw")
                        for t in range(NT):
                            nc.tensor.matmul(
                                ptp[:, t * 128:(t + 1) * 128],
                                c['au'][:, t * 128:(t + 1) * 128], idh[:],
                                is_transpose=True, start=True, stop=True,
                                skip_group_check=True)
                        c['auT'] = patT.tile([128, NPAD], f16, tag="auT", name="auT")
                        evac(c['auT'][:], ptp[:], fast=True)
                    return u

                def mk_seg3(hp):
                    col = 2 * hp * DH
                    c = ch[hp]
                    def u():
                        # rep_delta (pair-fused, diag blocks valid)
                        prd = psml.tile([128, 128], f32, tag="sm", name="sm")
                        for t in range(NT):
                            nc.tensor.matmul(
                                prd[:], c['auT'][:, t * 128:(t + 1) * 128],
                                st['pnat'][t][:, col:col + 128],
                                start=(t == 0), stop=(t == NT - 1),
                            )
                        # reps' = reps + (step_rep * rcp) * rep_delta
                        svec = psm.tile([128, 1], f32, tag="svec", name="svec")
                        nc.vector.tensor_scalar(
                            svec[:], c['rcp'][:], srp[:, hp:hp + 1], None,
                            op0=mybir.AluOpType.mult)
                        c['ru'] = psm.tile([128, DH], f16, tag="ru", name="ru")
                        ru = c['ru']
                        for p0, p1, q0 in ((0, 64, 0), (64, 128, 64)):
                            dve_work(190)
                            nc.vector.tensor_scalar(
                                ru[p0:p1, :], prd[p0:p1, q0:q0 + DH],
                                svec[p0:p1, :], None, op0=mybir.AluOpType.mult)
                            nc.gpsimd.tensor_add(
                                ru[p0:p1, :], ru[p0:p1, :],
                                st['rstk'][p0:p1, col:col + DH])
                    return u

                def mk_seg4(hp):
                    c = ch[hp]
                    def u():
                        ru = c['ru']
                        # reps'^T
                        prT2 = psml.tile([128, DH], f16, tag="sm", name="sm")
                        nc.tensor.matmul(prT2[0:64, :], ru[0:64, :],
                                         idh[0:64, 0:64], is_transpose=True,
                                         start=True, stop=True,
                                         skip_group_check=True)
                        nc.tensor.matmul(prT2[64:128, :], ru[64:128, :],
                                         idh[64:128, 64:128], is_transpose=True,
                                         start=True, stop=True,
                                         skip_group_check=True)
                        ruT = psm.tile([128, DH], f16, tag="ruT", name="ruT")
                        evac(ruT[:], prT2[:], fast=True)

                        # attn2_u = exp(SCALE * reps' @ reps'^T)  (symmetric)
                        pl2 = psml.tile([128, DH], f32, tag="sm", name="sm")
                        for p0, p1 in ((0, 64), (64, 128)):
                            nc.tensor.matmul(pl2[p0:p1, :], ruT[p0:p1, :],
                                             ruT[p0:p1, :], start=True,
                                             stop=True, skip_group_check=True)
                        a2 = psm.tile([128, DH], f16, tag="a2", name="a2")
                        d3 = psm.tile([128, 1], f32, tag="d3", name="d3")
                        act_work(64 * 0.833 + 320)
                        nc.scalar.activation(a2[:], pl2[:], AF.Exp, scale=SCALE,
                                             accum_out=d3[:])
                        rcp2 = psm.tile([128, 1], f32, tag="rcp2", name="rcp2")
                        nc.vector.reciprocal(rcp2[:], d3[:])

                        # rtr = attn2_u @ reps',  scaled by rcp*rcp2*step_x
                        prtr = psml.tile([128, DH], f32, tag="sm", name="sm")
                        for p0, p1 in ((0, 64), (64, 128)):
                            nc.tensor.matmul(prtr[p0:p1, :], a2[p0:p1, :],
                                             ru[p0:p1, :], start=True,
                                             stop=True, skip_group_check=True)
                        s2 = psm.tile([128, 1], f32, tag="s2", name="s2")
                        dve_work(300)
                        nc.vector.tensor_scalar(
                            s2[:], rcp2[:], c['rcp'][:], sxp[:, hp:hp + 1],
                            op0=mybir.AluOpType.mult, op1=mybir.AluOpType.mult)
                        c['rtrs'] = psm.tile([128, DH], f16, tag="rtrs", name="rtrs")
                        nc.vector.tensor_scalar(
                            c['rtrs'][:], prtr[:], s2[:], None,
                            op0=mybir.AluOpType.mult)
                    return u

                def mk_seg5(hp):
                    c = ch[hp]
                    def u():
                        # delta_tokens^T [d-pair, n] (two 1-bank psum chunks)
                        for c0, c1 in ((0, 512), (512, NPATCH)):
                            pdt = ppj.tile([128, 512], f32, tag="proj", name="proj")
                            for p0, p1 in ((0, 64), (64, 128)):
                                nc.tensor.matmul(
                                    pdt[p0:p1, 0:c1 - c0], c['rtrs'][p0:p1, :],
                                    c['au'][p0:p1, c0:c1], start=True,
                                    stop=True, skip_group_check=True)
                            evac(st['dT8'][:, hp, 1 + c0:1 + c1], pdt[:, 0:c1 - c0])
                        nc.gpsimd.tensor_copy(st['dT8'][:, hp, 0:1],
                                               st['inT'][hp][:, 0:1])
                    return u

                for mk in (mk_seg0, mk_seg1, mk_seg2, mk_seg3, mk_seg4, mk_seg5):
                    for hp in range(KT):
                        units.append(mk(hp))
                return units

            def out_units(st):
                """y^T = W_out^T @ deltaT + x^T (pre-loaded in yst), one DMA."""
                units = []

                def mk_out(m, c0, c1):
                    def u():
                        ps = ppj.tile([128, 512], f32, tag="proj", name="proj")
                        for k in range(0, KT, 2):
                            nc.tensor.matmul(
                                ps[:, 0:c1 - c0],
                                wout_sb[:, k:k + 2, m * 128:(m + 1) * 128],
                                st['dT8'][:, k:k + 2, c0:c1],
                                start=(k == 0), stop=(k == KT - 2),
                                perf_mode=DR)
                        ysl = st['yst'][:, m * NP + c0:m * NP + c1]
                        dve_work((c1 - c0) * 1.04 + 250)
                        nc.vector.tensor_add(ysl, ps[:, 0:c1 - c0], ysl)
                    return u
                for m in range(KT):
                    for c0, c1 in ((0, 512), (512, NP)):
                        units.append(mk_out(m, c0, c1))

                def u_store():
                    nc.gpsimd.dma_start(yT_d[st['s']], st['yst'][:])
                units.append(u_store)
                return units

            # ---------------- pipelined emission ----------------

            def interleave(a, b):
                """Alternate units from two lists proportionally."""
                out = []
                na, nb = len(a), len(b)
                ia = ib = 0
                while ia < na or ib < nb:
                    if ia * max(nb, 1) <= ib * max(na, 1):
                        if ia < na:
                            out.append(a[ia]); ia += 1
                        else:
                            out.append(b[ib]); ib += 1
                    else:
                        if ib < nb:
                            out.append(b[ib]); ib += 1
                        else:
                            out.append(a[ia]); ia += 1
                return out

            states = [dict(s=s) for s in range(BS)]
            for i in range(BS + 1):
                P = proj_units(states[i]) if i < BS else []
                A = attn_units(states[i - 1]) if i >= 1 else []
                O = out_units(states[i - 1]) if i >= 1 else []
                if i == 0:
                    seq = P
                else:
                    seq = interleave(A, P) + O
                for u in seq:
                    u()

    nc.finalize()
    return nc


def _get_state():
    if "nc" not in _STATE:
        _STATE["nc"] = _build_program()
    return _STATE["nc"]


def _host_inputs(x, W_in, W_out, step_x, step_rep):
    Ph = _pool_matrix(GRID, REP)
    Pfull = np.kron(Ph, Ph)                      # [64, 784]
    pft = np.zeros((NPAD, DH), np.float16)
    pft[:NPATCH, :] = Pfull.T.astype(np.float16)

    srp = np.zeros((128, 6), np.float32)
    sxp = np.zeros((128, 6), np.float32)
    sr = np.asarray(step_rep).reshape(HEADS)
    sx = np.asarray(step_x).reshape(HEADS)
    for hp in range(6):
        srp[0:64, hp] = sr[2 * hp]
        srp[64:128, hp] = sr[2 * hp + 1]
        sxp[0:64, hp] = sx[2 * hp]
        sxp[64:128, hp] = sx[2 * hp + 1]

    import ml_dtypes
    f8 = ml_dtypes.float8_e4m3

    def pack_w(W):  # [D, D] -> [128, KT, D] fp8 (row k*128+p -> [p, k])
        return np.ascontiguousarray(
            np.transpose(np.asarray(W, np.float32).reshape(KT, 128, D),
                         (1, 0, 2))).astype(f8)

    com = {
        "win": pack_w(W_in),
        "wout": pack_w(W_out),
        "pft": pft,
        "idh": np.eye(128, dtype=np.float16),
        "srp": srp,
        "sxp": sxp,
    }
    # x -> partition-major tiled layout [B, 128, KT, NP]
    xT = np.zeros((B, KT, 128, NP), np.float32)
    xT[:, :, :, 0:N] = np.transpose(x, (0, 2, 1)).reshape(B, KT, 128, N)
    xT = np.ascontiguousarray(np.transpose(xT, (0, 2, 1, 3)))
    x8 = np.zeros((B, 128, KT, NP8), f8)
    x8[:, :, :, 0:NP] = xT.astype(f8)
    xT = xT.reshape(B, 128, KT * NP)
    in_maps = []
    for c in range(NC):
        m = dict(com)
        m["xT"] = xT[c * BS:(c + 1) * BS]
        m["x8"] = x8[c * BS:(c + 1) * BS]
        in_maps.append(m)
    return in_maps


def _unpack_y(yT):
    """[BS, 128, KT*NP] per-core output -> [BS, N, D] float32."""
    y4 = yT.reshape(BS, 128, KT, NP)
    yD = np.transpose(y4, (0, 2, 1, 3)).reshape(BS, D, NP)[:, :, 0:N]
    return np.ascontiguousarray(np.transpose(yD, (0, 2, 1)))


def kernel(x, W_in, W_out, step_x, step_rep):
    from concourse import bass2jax
    nc = _get_state()
    in_maps = _host_inputs(x, W_in, W_out, step_x, step_rep)
    res = bass2jax.run_bass_via_pjrt(nc, in_maps, NC)
    y = np.empty((B, N, D), np.float32)
    for c in range(NC):
        y[c * BS:(c + 1) * BS] = _unpack_y(np.asarray(res[c]["yT"]))
    return y
